# revision 19
# baseline (speedup 1.0000x reference)
"""Trainium2 Bass kernel for nn_Aggregation (SAN-style local aggregation).

out[n, g*32+cc, h, w] = sum_{kh,kw} input[n, g*32+cc, h-3+kh, w-3+kw] * weight[n, cc, kh*7+kw, h, w]

Sharding: data-parallel over batch N=16 across 8 NeuronCores (2 images/core).

Mode "v2" (default): per-core layout
  partition p = cc*4 + blk   (cc in [0,32): weight channel, blk in [0,4): block of 8 output rows)
  in_nc[p][n, g, rho, w]: rho = 1 + r, r in [0,14) the padded window rows
    (lead row rho=0 and spare row rho=15 absorb column spill), w unpadded.
  Weight is host-permuted to [n, (cc blk), kw, kh, hb, w] with the edge
  output-columns of each kw-tap zeroed host-side (exact: those weights
  multiply conv padding zeros in the reference), so products computed at
  spilled columns are zero.
  DVE computes one fat tensor_tensor per (kw, n): free dims (kh:7, g:8,
  (hb w):256) -- 3 free dims, the TENSOR3D ISA limit; per-instruction
  overhead (~950ns) is amortized over 14336 elems. GPSIMD takes 2 of the
  14 fat multiplies. Products are fp16; the Tensor engine accumulates all
  taps into an fp32 PSUM accumulator via identity matmuls; two fat-prod
  pairs are pre-summed on the DVE to offload the PE.
"""

import numpy as np

N, C, H, W = 16, 256, 32, 32
K, PAD = 7, 3
CC, G = 32, 8
KK = K * K
NCORES = 8
NPC = N // NCORES
BLK, HB = 4, 8
R = 14  # padded window rows per blk
ROWS = 16  # lead pad + 14 + spare
COLP = W + 2 * PAD

MODE = "v2"
_GPS_FATS = ()  # (kw, n) fat-mults offloaded to gpsimd
_MERGES = ()  # ((dst_kwn, src_kwn), ...): dst prod += src prod on DVE
_PROD_BUFS = 4
_DEBUG_STAGE = False

_cache = {}


def _build_v2():
    import concourse.bacc as bacc
    import concourse.mybir as mybir
    import concourse.tile as tile
    from concourse.bass import AP

    fp32 = mybir.dt.float32
    fp16 = mybir.dt.float16
    mult = mybir.AluOpType.mult
    add = mybir.AluOpType.add

    nc = bacc.Bacc("TRN2", target_bir_lowering=False, debug=False, num_devices=NCORES)
    x = nc.dram_tensor("input", [NPC, C, H, W], fp32, kind="ExternalInput").ap()
    # host-permuted weight: [n, cc*blk, kw, kh, hb, w]
    wt = nc.dram_tensor(
        "wt", [NPC, 128, K, K, HB, W], fp32, kind="ExternalInput"
    ).ap()
    idn = nc.dram_tensor("identity", [128, 128], fp16, kind="ExternalInput").ap()
    y = nc.dram_tensor("output", [NPC, C, H, W], fp32, kind="ExternalOutput").ap()
    if _DEBUG_STAGE:
        dbg_in = nc.dram_tensor(
            "dbg_in", [128, NPC * G * ROWS * W], fp16, kind="ExternalOutput"
        ).ap()
        dbg_w = nc.dram_tensor(
            "dbg_w", [128, NPC * K * K * HB * W], fp16, kind="ExternalOutput"
        ).ap()
        dbg_p = nc.dram_tensor(
            "dbg_p", [128, K * G * HB * W], fp16, kind="ExternalOutput"
        ).ap()

    IN_PITCH = NPC * G * ROWS * W  # 16384 per-partition elems of in_nc
    W_PITCH = NPC * K * K * HB * W  # 25088

    # (kw, n) fats on gpsimd; the rest on DVE
    GPS = list(_GPS_FATS)
    DVE_ORDER = [
        (3, 0), (3, 1), (2, 0), (4, 0), (2, 1), (4, 1),
        (1, 0), (5, 0), (1, 1), (5, 1),
    ] + [kn for kn in [(0, 0), (0, 1)] if kn not in GPS] + [(6, 0), (6, 1)]
    # pairs merged on DVE before the PE sees them: (a, b) -> b += a
    MERGE = dict(_MERGES)
    # PE consumption order (merged-away prods excluded)
    merged_away = set(MERGE.values())
    PE_ORDER = [kn for kn in DVE_ORDER if kn not in merged_away]
    if GPS:
        # gpsimd prods are ready mid-stream; consume them late but not last
        PE_ORDER = [kn for kn in PE_ORDER if kn not in GPS]
        PE_ORDER = PE_ORDER[:-2] + GPS + PE_ORDER[-2:]

    with tile.TileContext(nc) as tc:
        with (
            tc.tile_pool(name="main", bufs=1) as pool,
            tc.tile_pool(name="prod", bufs=_PROD_BUFS) as ppool,
            tc.tile_pool(name="psum", bufs=1, space="PSUM") as pspool,
        ):
            in_nc = pool.tile([128, NPC, G, ROWS, W], fp16)
            w16 = pool.tile([128, NPC, K, K, HB, W], fp16)
            acc = pool.tile([128, NPC * G * HB * W], fp32)
            ident = pool.tile([128, 128], fp16)
            acc_ps = pspool.tile([128, NPC * G * HB * W], fp32)

            nc.sync.dma_start(out=ident[:], in_=idn[:])
            # touch ACT so its table set loads during the DMA wait
            warm = pool.tile([128, 1], fp32)
            nc.scalar.copy(out=warm[:], in_=ident[:, 0:1])

            # row halos: blk0 top (r 0:3 -> rho 1:4), blk3 bottom (r 11:14 ->
            # rho 12:15). Engine APs cannot stride partitions, so zero those
            # rows on ALL partitions; the input DMAs then overwrite the
            # non-halo parts (WAW, ordered by the tile framework).
            # include lead row 0 and spare row 15: they are read via column
            # spill with zero weights, and 0*NaN = NaN if left uninitialized
            nc.vector.memset(in_nc[:, :, :, 0:4, :], 0.0)
            nc.vector.memset(in_nc[:, :, :, 12:16, :], 0.0)

            # input cast-DMAs, one per (n, blk): all valid rows at once
            for n in range(NPC):
                for blk in range(BLK):
                    h0 = max(0, blk * HB - PAD)
                    h1 = min(H, blk * HB - PAD + R)
                    r0 = h0 - (blk * HB - PAD)
                    dst = in_nc[blk::BLK, n, :, 1 + r0 : 1 + r0 + (h1 - h0), :]
                    src = x[n].rearrange("(g cc) h w -> cc g h w", g=G)[:, :, h0:h1]
                    nc.gpsimd.dma_start(out=dst, in_=src)
                if n == 0:
                    _w_dma(nc, AP, wt, w16, 3, 0, W_PITCH)
            _w_dma(nc, AP, wt, w16, 3, 1, W_PITCH)
            for kw, n in [(0, 0), (2, 0), (4, 0), (0, 1), (2, 1), (4, 1),
                          (1, 0), (5, 0), (1, 1), (5, 1), (6, 0), (6, 1)]:
                _w_dma(nc, AP, wt, w16, kw, n, W_PITCH)

            def fat_aps(kw, n, pb):
                v = in_nc[:]
                in0 = AP(
                    v.tensor,
                    v.offset + n * G * ROWS * W + (ROWS - R - 1) * W - PAD + kw,
                    [[IN_PITCH, 128], [W, K], [ROWS * W, G], [1, HB * W]],
                )
                wv = w16[:]
                in1 = AP(
                    wv.tensor,
                    wv.offset + n * K * K * HB * W + kw * K * HB * W,
                    [[W_PITCH, 128], [HB * W, K], [0, G], [1, HB * W]],
                )
                po = pb[:]
                outp = AP(
                    po.tensor,
                    po.offset,
                    [[K * G * HB * W, 128], [G * HB * W, K], [HB * W, G], [1, HB * W]],
                )
                return in0, in1, outp

            prods = {}
            dbg_prod = {}

            def emit_fat(kw, n, eng, pb=None):
                # gpsimd prods live in dedicated tiles (consumed late by the
                # PE; keeping them in the rotating pool would deadlock it)
                if pb is None:
                    pb = ppool.tile([128, K, G * HB * W], fp16)
                in0, in1, outp = fat_aps(kw, n, pb)
                eng.tensor_tensor(out=outp, in0=in0, in1=in1, op=mult)
                prods[(kw, n)] = pb
                if _DEBUG_STAGE and (kw, n) == (3, 0) and not dbg_prod:
                    dbg_prod[0] = True
                    nc.sync.dma_start(
                        out=dbg_p, in_=pb[:].rearrange("p k f -> p (k f)")
                    )

            # gpsimd fats first in its program order (after its DMAs)
            for kw, n in GPS:
                gpb = pool.tile([128, K, G * HB * W], fp16, tag=f"gps{kw}_{n}")
                emit_fat(kw, n, nc.gpsimd, pb=gpb)

            pass_idx = {0: 0, 1: 0}
            NPASS = len(PE_ORDER) // NPC

            def pe_pass(kw, n):
                pb = prods[(kw, n)]
                pf = pb[:].rearrange("p k f -> p (k f)")
                i = pass_idx[n]
                for kh in range(K):
                    for b in range(4):
                        nc.tensor.matmul(
                            out=acc_ps[:, n * 2048 + b * 512 : n * 2048 + (b + 1) * 512],
                            lhsT=ident[:],
                            rhs=pf[:, kh * 2048 + b * 512 : kh * 2048 + (b + 1) * 512],
                            start=(i == 0 and kh == 0),
                            stop=(i == NPASS - 1 and kh == K - 1),
                        )
                pass_idx[n] += 1

            emitted = 0
            for kw, n in DVE_ORDER:
                emit_fat(kw, n, nc.vector)
                if (kw, n) in MERGE:
                    src = prods[MERGE[(kw, n)]]
                    dst = prods[(kw, n)]
                    nc.vector.tensor_tensor(
                        out=dst[:], in0=dst[:], in1=src[:], op=add
                    )
                # interleave PE passes with DVE emission in PE_ORDER as their
                # prods become available in program order
                while emitted < len(PE_ORDER):
                    pkw, pn = PE_ORDER[emitted]
                    if (pkw, pn) not in prods:
                        break
                    if (pkw, pn) in [MERGE.get(k) for k in MERGE]:
                        # merged-away prod: consumed via its partner
                        emitted += 1
                        continue
                    pe_pass(pkw, pn)
                    emitted += 1

            if _DEBUG_STAGE:
                nc.sync.dma_start(
                    out=dbg_in, in_=in_nc[:].rearrange("p n g r w -> p (n g r w)")
                )
                nc.sync.dma_start(
                    out=dbg_w, in_=w16[:].rearrange("p n a b h w -> p (n a b h w)")
                )

            # evict PSUM -> SBUF in per-n halves, quarters alternating
            # DVE/ACT, stores per (n, g) right after their quarter
            for n in range(NPC):
                for q in range(4):
                    eng = nc.vector.tensor_copy if q % 2 == 0 else nc.scalar.copy
                    lo = n * 2048 + q * 512
                    eng(out=acc[:, lo : lo + 512], in_=acc_ps[:, lo : lo + 512])
                    for g in (2 * q, 2 * q + 1):
                        dsty = y[n].rearrange(
                            "(g cc) (blk hb) w -> g cc blk (hb w)", g=G, blk=BLK
                        )
                        deng = nc.sync if g % 2 == 0 else nc.scalar
                        deng.dma_start(
                            out=dsty[g],
                            in_=acc[:, n * 2048 + g * 256 : n * 2048 + (g + 1) * 256],
                        )

    nc.compile()
    return nc


def _w_dma(nc, AP, wt, w16, kw, n, W_PITCH):
    K_, HB_, W_ = 7, 8, 32
    dst = AP(
        w16[:].tensor,
        w16[:].offset + n * K_ * K_ * HB_ * W_ + kw * K_ * HB_ * W_,
        [[W_PITCH, 128], [HB_ * W_, K_], [1, HB_ * W_]],
    )
    src = AP(
        wt.tensor,
        n * 128 * K_ * K_ * HB_ * W_ + kw * K_ * HB_ * W_,
        [[K_ * K_ * HB_ * W_, 128], [HB_ * W_, K_], [1, HB_ * W_]],
    )
    nc.gpsimd.dma_start(out=dst, in_=src)


def _get_nc(mode=None):
    mode = mode or MODE
    if mode not in _cache:
        if mode == "v2":
            _cache[mode] = _build_v2()
        else:
            raise ValueError(mode)
    return _cache[mode]


def _prep_weight(weight):
    # [N, CC, KK, H, W] -> [N, (cc blk), kw, kh, hb, w], edge out-columns of
    # each kw zeroed (exact: they multiply conv-padding zeros)
    w = weight.reshape(N, CC, K, K, BLK, HB, W)  # [n, cc, kh, kw, blk, hb, w]
    w = np.ascontiguousarray(w.transpose(0, 1, 4, 3, 2, 5, 6))
    # -> [n, cc, blk, kw, kh, hb, w]
    for kw in range(K):
        if kw < PAD:
            w[:, :, :, kw, :, :, 0 : PAD - kw] = 0.0
        elif kw > PAD:
            w[:, :, :, kw, :, :, W + PAD - kw : W] = 0.0
    return w.reshape(N, 128, K, K, HB, W)


def kernel(input_, weight, _trace=False, _mode=None):
    from concourse.bass_utils import run_bass_kernel_spmd

    nc = _get_nc(_mode)
    input_ = np.ascontiguousarray(input_, dtype=np.float32)
    weight = np.ascontiguousarray(weight, dtype=np.float32)
    wt = _prep_weight(weight)
    eye = np.eye(128, dtype=np.float16)
    in_maps = [
        {
            "input": input_[i * NPC : (i + 1) * NPC],
            "wt": wt[i * NPC : (i + 1) * NPC],
            "identity": eye,
        }
        for i in range(NCORES)
    ]
    res = run_bass_kernel_spmd(nc, in_maps, list(range(NCORES)), trace=_trace)
    _cache["last_result"] = res
    out = np.concatenate([res.results[i]["output"] for i in range(NCORES)], axis=0)
    return out
